# revision 17
# baseline (speedup 1.0000x reference)
"""Trainium2 Bass kernel for nn_Attention_9887014715893.

Multi-head attention forward (B=1, S=4096, D=1024, H=16, E=64, fp32):
    qkv = x @ w_qkv ; q,k,v per head ; softmax(q k^T / 8 + mask) @ v

Sharding: tensor-parallel over heads. 8 cores x 2 heads each. Each core gets
the full x (transposed + bf16-cast on host) and its own 128-column bf16
slices of w_qkv, and produces out[:, 128c:128c+128]. No collectives.

Engine budget per core: 33.5M exps keep ACT busy ~264us (the bottleneck);
PE needs ~260us of bf16 matmul streaming (fp32 matmuls would run the PE at
the 1.2GHz "others" clock - bf16 runs 2.4GHz). So everything is organized
around keeping ACT 100% fed:
  - all matmul operands bf16 (PSUM accumulation fp32); host pre-casts inputs
  - scores tiles are [128, 1536] fp32 (3 PSUM banks; 2 slots + 2 acc banks
    = 8 banks exactly) so each ACTIVATE covers 1536 elements, amortizing the
    ~260ns/instruction access latency
  - x/w DMAs split across both HWDGE queues (sync + scalar)
  - projection is woven into early attention units at sub-microsecond
    granularity so ACT starts ~16us in and rarely starves

Layouts: QT2/KT2 [128, 4096] bf16 (two heads stacked on partitions,
1/sqrt(E) folded into wq on host). V projected directly transposed
(out[s,e] = x-tile^T @ wv) into va [128, 65*32] bf16 with a ones column so
the softmax denominator falls out of the attn@V matmul as row 64. Scores
kept transposed (k on partitions, q free). Epilogue: raw [65, q]
accumulators DMA'd to HBM; the divide by the denominator row and the final
[e,s]->[s,e] transpose happen on the host during the gather.
"""

import sys

if "/opt/trn_rl_repo" not in sys.path:
    sys.path.insert(0, "/opt/trn_rl_repo")

import numpy as np
from contextlib import ExitStack

import concourse.bass as bass
import concourse.bacc as bacc
import concourse.tile as tile
import concourse.mybir as mybir
from concourse.bass_utils import run_bass_kernel_spmd

F32 = mybir.dt.float32
BF16 = mybir.dt.bfloat16
EXP = mybir.ActivationFunctionType.Exp

S = 4096          # sequence length
DM = 1024         # model dim
E = 64            # head dim
NCORES = 8
EC = 128          # output columns per core (2 heads x 64)
QC = 512          # q chunk (free axis of transposed scores)
NQ = S // QC      # 8 q chunks
NK = S // 128     # 32 k tiles
ND = DM // 128    # 8 d tiles
# attention "units": u 0..9 cover 3 k-tiles each, u 10 covers the last 2
NU = 11


def _unit_kts(u):
    return range(3 * u, min(3 * u + 3, NK))


def _build_kernel(with_mask: bool):
    nc = bacc.Bacc("TRN2", target_bir_lowering=False, debug=False,
                   enable_asserts=False, num_devices=NCORES)
    xT = nc.dram_tensor("xT", [DM, S], BF16, kind="ExternalInput").ap()
    wq = nc.dram_tensor("wq", [DM, EC], BF16, kind="ExternalInput").ap()
    wk = nc.dram_tensor("wk", [DM, EC], BF16, kind="ExternalInput").ap()
    wv = nc.dram_tensor("wv", [DM, EC], BF16, kind="ExternalInput").ap()
    if with_mask:
        maskT = nc.dram_tensor("maskT", [S, S], BF16, kind="ExternalInput").ap()
    # raw transposed output: rows 0-64 head0 {outT | denom}, 65-129 head1.
    outT = nc.dram_tensor("outT", [130, S], F32, kind="ExternalOutput").ap()

    with tile.TileContext(nc) as tc, ExitStack() as ctx:
        w_pool = ctx.enter_context(tc.tile_pool(name="w", bufs=1))
        wq_sb = w_pool.tile([128, DM], BF16)
        wk_sb = w_pool.tile([128, DM], BF16)
        wv_sb = w_pool.tile([128, DM], BF16)
        # one 3D-AP DMA per weight: dst[p, 128t+j] = w[128t+p, j]
        # (k first on sync, v on the scalar queue -> K proj starts earliest)
        for eng, wsb, w in ((nc.sync, wk_sb, wk), (nc.scalar, wv_sb, wv),
                            (nc.sync, wq_sb, wq)):
            eng.dma_start(
                wsb[:].rearrange("p (t j) -> p t j", t=ND),
                w.rearrange("(t p) j -> p t j", t=ND))

        qt_pool = ctx.enter_context(tc.tile_pool(name="qt", bufs=1))
        QT2 = qt_pool.tile([128, S], BF16)   # rows 0-63 head0 e-dims, 64-127 head1
        KT2 = qt_pool.tile([128, S], BF16)
        va_pool = ctx.enter_context(tc.tile_pool(name="va", bufs=1))
        va = [va_pool.tile([128, 65 * NK], BF16, name=f"va{h}") for h in range(2)]
        ones_b = va_pool.tile([128, 1], BF16)
        nc.vector.memset(ones_b[:], 1.0)
        for h in range(2):
            nc.vector.tensor_copy(va[h][:, 64:65 * NK:65],
                                  ones_b[:].to_broadcast([128, NK]))

        # full x resident in SBUF (8 chunks x [128, 8*512] bf16 = 64KB/par)
        xs_pool = ctx.enter_context(tc.tile_pool(name="xs", bufs=1))
        xs = [xs_pool.tile([128, ND * QC], BF16, name=f"xs{c}") for c in range(NQ)]

        # PSUM: psA 2 slots x 3 banks (scores + proj psums), psB 2 x 1 bank
        psA = ctx.enter_context(tc.tile_pool(name="psA", bufs=2, space="PSUM"))
        psB = ctx.enter_context(tc.tile_pool(name="psB", bufs=2, space="PSUM"))

        exp_pool = ctx.enter_context(tc.tile_pool(name="exp", bufs=6))
        accsb_pool = ctx.enter_context(tc.tile_pool(name="accsb", bufs=4))
        if with_mask:
            msk_pool = ctx.enter_context(tc.tile_pool(name="msk", bufs=3))

        def dma_chunk(c):
            # alternate d-tiles across the two HWDGE queues (sync / scalar)
            s0 = QC * c
            for t in range(ND):
                eng = nc.sync if t % 2 == 0 else nc.scalar
                eng.dma_start(xs[c][:, QC * t:QC * (t + 1)],
                              xT[128 * t:128 * (t + 1), s0:s0 + QC])

        def proj_qk(wsb, dst, c, ps=None, t_range=None):
            # t_range splits the 8 accumulation matmuls across call sites
            # (sub-1.7us PE bursts keep ACT's short psA runway alive); pass
            # the returned psum tile to the continuation call.
            s0 = QC * c
            if ps is None:
                ps = psA.tile([128, QC], F32, tag="psA")
            ts = list(range(ND)) if t_range is None else list(t_range)
            for t in ts:
                nc.tensor.matmul(ps[:], lhsT=wsb[:, 128 * t:128 * (t + 1)],
                                 rhs=xs[c][:, QC * t:QC * (t + 1)],
                                 start=(t == 0), stop=(t == ND - 1))
            if ts[-1] == ND - 1:
                nc.vector.tensor_copy(dst[:, s0:s0 + QC], ps[:])
            return ps

        def proj_v_tile(c, st):
            # direct transposed V: out[s-tile, e] = sum_t x-tile^T @ wv-tile
            kk = 4 * c + st
            ps = psA.tile([128, 128], F32, tag="psA")
            for t in range(ND):
                nc.tensor.matmul(
                    ps[:],
                    lhsT=xs[c][:, QC * t + 128 * st:QC * t + 128 * (st + 1)],
                    rhs=wv_sb[:, 128 * t:128 * (t + 1)],
                    start=(t == 0), stop=(t == ND - 1))
            nc.vector.tensor_copy(va[0][:, 65 * kk:65 * kk + 64], ps[:, 0:64])
            nc.vector.tensor_copy(va[1][:, 65 * kk:65 * kk + 64], ps[:, 64:128])

        # ---- attention unit: 3 (or 2) k-tiles for (qc, h), one ACTIVATE ----
        def attn_scexp(qc, h, u, ex):
            q0 = QC * qc
            kts = list(_unit_kts(u))
            w = 512 * len(kts)
            if with_mask:
                msk = msk_pool.tile([128, 1536], BF16, tag="msk")
                for j, kt in enumerate(kts):
                    nc.sync.dma_start(
                        msk[:, 512 * j:512 * (j + 1)],
                        maskT[128 * kt:128 * (kt + 1), q0:q0 + QC])
            sc = psA.tile([128, 1536], F32, tag="psA", name=f"sc{qc}_{h}_{u}")
            for j, kt in enumerate(kts):
                nc.tensor.matmul(
                    sc[:, 512 * j:512 * (j + 1)],
                    lhsT=KT2[64 * h:64 * (h + 1), 128 * kt:128 * (kt + 1)],
                    rhs=QT2[64 * h:64 * (h + 1), q0:q0 + QC],
                    start=True, stop=True,
                    tile_position=(64 * h, 0),
                )
            if with_mask:
                nc.vector.tensor_tensor(out=sc[:, 0:w], in0=sc[:, 0:w],
                                        in1=msk[:, 0:w], op=mybir.AluOpType.add)
            nc.scalar.activation(ex[:, 0:w], sc[:, 0:w], EXP)

        def attn_acc(h, u, ex, accs):
            for j, kt in enumerate(_unit_kts(u)):
                nc.tensor.matmul(
                    accs[:],
                    lhsT=va[h][:, 65 * kt:65 * kt + 65],
                    rhs=ex[:, 512 * j:512 * (j + 1)],
                    start=(kt == 0), stop=(kt == NK - 1),
                )

        def attn_unit(qc, h, u, accs):
            ex = exp_pool.tile([128, 1536], BF16, tag="exp", name=f"ex{qc}_{h}_{u}")
            attn_scexp(qc, h, u, ex)
            attn_acc(h, u, ex, accs)

        def epilogue(qc, h, accs):
            asb = accsb_pool.tile([65, QC], F32, tag="accsb")
            nc.vector.tensor_copy(asb[:], accs[:])
            nc.sync.dma_start(outT[65 * h:65 * h + 65, QC * qc:QC * (qc + 1)], asb[:])

        # deferred-exp store for q1/h0's units computed during the proj phase
        exd_pool = ctx.enter_context(tc.tile_pool(name="exd", bufs=1))
        exd = [exd_pool.tile([128, 1536], BF16, name=f"exd{u}")
               for u in range(NU)]

        # ---------------- emission ----------------
        # Proj phase: weave three streams of attention units between
        # projection sub-bursts as their k-tiles become ready, so ACT (the
        # bottleneck) starts early and rarely starves while PE does the
        # 41us of projection work:
        #   streams 0,1 = (q0,h0) / (q0,h1), acc matmuls inline (2 PSUM accs)
        #   stream  2   = (q1,h0), exp parked in exd[]; its acc matmuls run
        #                 after q0's epilogues free a PSUM accumulator bank.
        for c in range(3):
            dma_chunk(c)
        accs0 = [psB.tile([65, QC], F32, tag="psB", name=f"acc0_{h}")
                 for h in range(2)]
        next_u = [0, 0, 0]

        def emit_units(u_lim, n_max=NU * 3, s2_lim=-1):
            # s2 (the deferred q1/h0 stream) is gated until QT2 chunk 1 is
            # projected (emitted at the start of proj chunk 1)
            lims = [u_lim, u_lim, min(u_lim, s2_lim)]
            n = 0
            while n < n_max and any(next_u[s] <= lims[s] for s in range(3)):
                for s in range(3):
                    if next_u[s] <= lims[s] and n < n_max:
                        if s < 2:
                            attn_unit(0, s, next_u[s], accs0[s])
                        else:
                            attn_scexp(1, 0, next_u[2], exd[next_u[2]])
                        next_u[s] += 1
                        n += 1

        for c in range(NQ):
            if c + 3 < NQ:
                dma_chunk(c + 3)
            proj_qk(wk_sb, KT2, c)
            if c == 0:
                proj_qk(wq_sb, QT2, 0)
            if c == 1:
                proj_qk(wq_sb, QT2, 1)
            for st in range(4):
                proj_v_tile(c, st)
                # unit u needs va k-tiles 3u..3u+2, i.e. all tiles <= 4c+st
                u_lim = (4 * c + st - 2) // 3
                emit_units(u_lim, NU * 3 if st == 3 else 2,
                           s2_lim=NU if c >= 2 else -1)
        emit_units(NU - 1, s2_lim=NU)
        epilogue(0, 0, accs0[0])
        epilogue(0, 1, accs0[1])

        # resolution: q1/h0's deferred acc matmuls, woven with q1/h1's units
        # (which keep ACT busy); Q proj for chunk 2 slips in as two bursts.
        accs10 = psB.tile([65, QC], F32, tag="psB", name="acc1_0")
        accs11 = psB.tile([65, QC], F32, tag="psB", name="acc1_1")
        qps = None
        for u in range(NU):
            attn_acc(0, u, exd[u], accs10)
            attn_unit(1, 1, u, accs11)
            if u == 3:
                qps = proj_qk(wq_sb, QT2, 2, t_range=range(0, 4))
            elif u == 4:
                proj_qk(wq_sb, QT2, 2, ps=qps, t_range=range(4, 8))
        epilogue(1, 0, accs10)
        epilogue(1, 1, accs11)

        # steady phase: (qc, h) blocks; Q proj for qc+1 in two 4-mm bursts
        for qc in range(2, NQ):
            for h in range(2):
                accs = psB.tile([65, QC], F32, tag="psB", name=f"acc{qc}_{h}")
                qps = None
                for u in range(NU):
                    attn_unit(qc, h, u, accs)
                    if h == 1 and qc < NQ - 1:
                        if u == 3:
                            qps = proj_qk(wq_sb, QT2, qc + 1, t_range=range(0, 4))
                        elif u == 4:
                            proj_qk(wq_sb, QT2, qc + 1, ps=qps,
                                    t_range=range(4, 8))
                epilogue(qc, h, accs)

    nc.compile()
    return nc


_CACHE: dict = {}


def _get_kernel(with_mask: bool):
    if with_mask not in _CACHE:
        _CACHE[with_mask] = _build_kernel(with_mask)
    return _CACHE[with_mask]


def _bf16(a):
    import ml_dtypes
    return np.ascontiguousarray(a).astype(ml_dtypes.bfloat16)


def build_in_maps(x, w_qkv, maskT=None):
    xTb = _bf16(x[0].T)                                    # [DM, S]
    scale = np.float32(1.0 / np.sqrt(E))
    in_maps = []
    for c in range(NCORES):
        m = {
            "xT": xTb,
            "wq": _bf16(w_qkv[:, EC * c:EC * (c + 1)] * scale),
            "wk": _bf16(w_qkv[:, DM + EC * c:DM + EC * (c + 1)]),
            "wv": _bf16(w_qkv[:, 2 * DM + EC * c:2 * DM + EC * (c + 1)]),
        }
        if maskT is not None:
            m["maskT"] = maskT
        in_maps.append(m)
    return in_maps


def kernel(x: np.ndarray, mask: np.ndarray, w_qkv: np.ndarray) -> np.ndarray:
    x = np.asarray(x, dtype=np.float32)
    mask = np.asarray(mask, dtype=np.float32)
    w_qkv = np.asarray(w_qkv, dtype=np.float32)
    assert x.shape == (1, S, DM) and w_qkv.shape == (DM, 3 * DM)

    with_mask = bool(np.any(mask))
    nc = _get_kernel(with_mask)

    maskT = None
    if with_mask:
        maskT = _bf16(np.broadcast_to(mask, (1, 1, S, S))[0, 0].T)
    in_maps = build_in_maps(x, w_qkv, maskT)

    res = run_bass_kernel_spmd(nc, in_maps, core_ids=list(range(NCORES)))
    # host-side normalize (softmax denominator is row 64/129) and transpose
    outs = []
    for c in range(NCORES):
        o = res.results[c]["outT"]                       # [130, S]
        h0 = o[0:64] / o[64:65]
        h1 = o[65:129] / o[129:130]
        outs.append(np.concatenate([h0, h1], axis=0).T)  # [S, 128]
    return np.ascontiguousarray(
        np.concatenate(outs, axis=1), dtype=np.float32).reshape(1, S, DM)


# revision 18
# speedup vs baseline: 1.1206x; 1.1206x over previous
"""Trainium2 Bass kernel for nn_Attention_9887014715893.

Multi-head attention forward (B=1, S=4096, D=1024, H=16, E=64, fp32):
    qkv = x @ w_qkv ; q,k,v per head ; softmax(q k^T / 8 + mask) @ v

Sharding: tensor-parallel over heads. 8 cores x 2 heads each. Each core gets
the full x (transposed + bf16-cast on host) and its own 128-column bf16
slices of w_qkv, and produces out[:, 128c:128c+128]. No collectives.

Engine budget per core: 33.5M exps keep ACT busy ~264us (the bottleneck);
PE needs ~260us of bf16 matmul streaming (fp32 matmuls would run the PE at
the 1.2GHz "others" clock - bf16 runs 2.4GHz). So everything is organized
around keeping ACT 100% fed:
  - all matmul operands bf16 (PSUM accumulation fp32); host pre-casts inputs
  - scores tiles are [128, 1024] fp32 (2 PSUM banks; 3 slots + 2 acc banks
    = 8 banks exactly): each ACTIVATE covers 1024 elements and the 3-slot
    ring gives ACT a ~2.2us lookahead buffer over PE hiccups
  - x/w DMAs split across both HWDGE queues (sync + scalar)
  - projection is woven into early attention units at sub-microsecond
    granularity so ACT starts ~16us in and rarely starves

Layouts: QT2/KT2 [128, 4096] bf16 (two heads stacked on partitions,
1/sqrt(E) folded into wq on host). V projected directly transposed
(out[s,e] = x-tile^T @ wv) into va [128, 65*32] bf16 with a ones column so
the softmax denominator falls out of the attn@V matmul as row 64. Scores
kept transposed (k on partitions, q free). Epilogue: raw [65, q]
accumulators DMA'd to HBM; the divide by the denominator row and the final
[e,s]->[s,e] transpose happen on the host during the gather.
"""

import sys

if "/opt/trn_rl_repo" not in sys.path:
    sys.path.insert(0, "/opt/trn_rl_repo")

import numpy as np
from contextlib import ExitStack

import concourse.bass as bass
import concourse.bacc as bacc
import concourse.tile as tile
import concourse.mybir as mybir
from concourse.bass_utils import run_bass_kernel_spmd

F32 = mybir.dt.float32
BF16 = mybir.dt.bfloat16
EXP = mybir.ActivationFunctionType.Exp

S = 4096          # sequence length
DM = 1024         # model dim
E = 64            # head dim
NCORES = 8
EC = 128          # output columns per core (2 heads x 64)
QC = 512          # q chunk (free axis of transposed scores)
NQ = S // QC      # 8 q chunks
NK = S // 128     # 32 k tiles
ND = DM // 128    # 8 d tiles
# attention "units": u 0..15 cover 2 k-tiles each
NU = 16


def _unit_kts(u):
    return range(2 * u, 2 * u + 2)


def _build_kernel(with_mask: bool):
    nc = bacc.Bacc("TRN2", target_bir_lowering=False, debug=False,
                   enable_asserts=False, num_devices=NCORES)
    xT = nc.dram_tensor("xT", [DM, S], BF16, kind="ExternalInput").ap()
    wq = nc.dram_tensor("wq", [DM, EC], BF16, kind="ExternalInput").ap()
    wk = nc.dram_tensor("wk", [DM, EC], BF16, kind="ExternalInput").ap()
    wv = nc.dram_tensor("wv", [DM, EC], BF16, kind="ExternalInput").ap()
    if with_mask:
        maskT = nc.dram_tensor("maskT", [S, S], BF16, kind="ExternalInput").ap()
    # raw transposed output: rows 0-64 head0 {outT | denom}, 65-129 head1.
    outT = nc.dram_tensor("outT", [130, S], F32, kind="ExternalOutput").ap()

    with tile.TileContext(nc) as tc, ExitStack() as ctx:
        w_pool = ctx.enter_context(tc.tile_pool(name="w", bufs=1))
        wq_sb = w_pool.tile([128, DM], BF16)
        wk_sb = w_pool.tile([128, DM], BF16)
        wv_sb = w_pool.tile([128, DM], BF16)
        # one 3D-AP DMA per weight: dst[p, 128t+j] = w[128t+p, j]
        # (k first on sync, v on the scalar queue -> K proj starts earliest)
        for eng, wsb, w in ((nc.sync, wk_sb, wk), (nc.scalar, wv_sb, wv),
                            (nc.sync, wq_sb, wq)):
            eng.dma_start(
                wsb[:].rearrange("p (t j) -> p t j", t=ND),
                w.rearrange("(t p) j -> p t j", t=ND))

        qt_pool = ctx.enter_context(tc.tile_pool(name="qt", bufs=1))
        QT2 = qt_pool.tile([128, S], BF16)   # rows 0-63 head0 e-dims, 64-127 head1
        KT2 = qt_pool.tile([128, S], BF16)
        va_pool = ctx.enter_context(tc.tile_pool(name="va", bufs=1))
        va = [va_pool.tile([128, 65 * NK], BF16, name=f"va{h}") for h in range(2)]
        ones_b = va_pool.tile([128, 1], BF16)
        nc.vector.memset(ones_b[:], 1.0)
        for h in range(2):
            nc.vector.tensor_copy(va[h][:, 64:65 * NK:65],
                                  ones_b[:].to_broadcast([128, NK]))

        # full x resident in SBUF (8 chunks x [128, 8*512] bf16 = 64KB/par)
        xs_pool = ctx.enter_context(tc.tile_pool(name="xs", bufs=1))
        xs = [xs_pool.tile([128, ND * QC], BF16, name=f"xs{c}") for c in range(NQ)]

        # PSUM: psA 3 slots x 2 banks (scores + proj psums), psB 2 x 1 bank
        psA = ctx.enter_context(tc.tile_pool(name="psA", bufs=3, space="PSUM"))
        psB = ctx.enter_context(tc.tile_pool(name="psB", bufs=2, space="PSUM"))

        exp_pool = ctx.enter_context(tc.tile_pool(name="exp", bufs=8))
        accsb_pool = ctx.enter_context(tc.tile_pool(name="accsb", bufs=4))
        if with_mask:
            msk_pool = ctx.enter_context(tc.tile_pool(name="msk", bufs=3))

        def dma_chunk(c):
            # alternate d-tiles across the two HWDGE queues (sync / scalar)
            s0 = QC * c
            for t in range(ND):
                eng = nc.sync if t % 2 == 0 else nc.scalar
                eng.dma_start(xs[c][:, QC * t:QC * (t + 1)],
                              xT[128 * t:128 * (t + 1), s0:s0 + QC])

        def proj_qk(wsb, dst, c, ps=None, t_range=None):
            # t_range splits the 8 accumulation matmuls across call sites
            # (sub-1.7us PE bursts keep ACT's short psA runway alive); pass
            # the returned psum tile to the continuation call.
            s0 = QC * c
            if ps is None:
                ps = psA.tile([128, QC], F32, tag="psA")
            ts = list(range(ND)) if t_range is None else list(t_range)
            for t in ts:
                nc.tensor.matmul(ps[:], lhsT=wsb[:, 128 * t:128 * (t + 1)],
                                 rhs=xs[c][:, QC * t:QC * (t + 1)],
                                 start=(t == 0), stop=(t == ND - 1))
            if ts[-1] == ND - 1:
                nc.vector.tensor_copy(dst[:, s0:s0 + QC], ps[:])
            return ps

        def proj_v_tile(c, st):
            # direct transposed V: out[s-tile, e] = sum_t x-tile^T @ wv-tile
            kk = 4 * c + st
            ps = psA.tile([128, 128], F32, tag="psA")
            for t in range(ND):
                nc.tensor.matmul(
                    ps[:],
                    lhsT=xs[c][:, QC * t + 128 * st:QC * t + 128 * (st + 1)],
                    rhs=wv_sb[:, 128 * t:128 * (t + 1)],
                    start=(t == 0), stop=(t == ND - 1))
            nc.vector.tensor_copy(va[0][:, 65 * kk:65 * kk + 64], ps[:, 0:64])
            nc.vector.tensor_copy(va[1][:, 65 * kk:65 * kk + 64], ps[:, 64:128])

        # ---- attention unit: 3 (or 2) k-tiles for (qc, h), one ACTIVATE ----
        def attn_scexp(qc, h, u, ex):
            q0 = QC * qc
            kts = list(_unit_kts(u))
            w = 512 * len(kts)
            if with_mask:
                msk = msk_pool.tile([128, 1024], BF16, tag="msk")
                for j, kt in enumerate(kts):
                    nc.sync.dma_start(
                        msk[:, 512 * j:512 * (j + 1)],
                        maskT[128 * kt:128 * (kt + 1), q0:q0 + QC])
            sc = psA.tile([128, 1024], F32, tag="psA", name=f"sc{qc}_{h}_{u}")
            for j, kt in enumerate(kts):
                nc.tensor.matmul(
                    sc[:, 512 * j:512 * (j + 1)],
                    lhsT=KT2[64 * h:64 * (h + 1), 128 * kt:128 * (kt + 1)],
                    rhs=QT2[64 * h:64 * (h + 1), q0:q0 + QC],
                    start=True, stop=True,
                    tile_position=(64 * h, 0),
                )
            if with_mask:
                nc.vector.tensor_tensor(out=sc[:, 0:w], in0=sc[:, 0:w],
                                        in1=msk[:, 0:w], op=mybir.AluOpType.add)
            nc.scalar.activation(ex[:, 0:w], sc[:, 0:w], EXP)

        def attn_acc(h, u, ex, accs):
            for j, kt in enumerate(_unit_kts(u)):
                nc.tensor.matmul(
                    accs[:],
                    lhsT=va[h][:, 65 * kt:65 * kt + 65],
                    rhs=ex[:, 512 * j:512 * (j + 1)],
                    start=(kt == 0), stop=(kt == NK - 1),
                )

        def attn_unit(qc, h, u, accs):
            ex = exp_pool.tile([128, 1024], BF16, tag="exp", name=f"ex{qc}_{h}_{u}")
            attn_scexp(qc, h, u, ex)
            attn_acc(h, u, ex, accs)

        def epilogue(qc, h, accs):
            asb = accsb_pool.tile([65, QC], F32, tag="accsb")
            nc.vector.tensor_copy(asb[:], accs[:])
            nc.sync.dma_start(outT[65 * h:65 * h + 65, QC * qc:QC * (qc + 1)], asb[:])

        # deferred-exp store for q1/h0's units computed during the proj phase
        exd_pool = ctx.enter_context(tc.tile_pool(name="exd", bufs=1))
        exd = [exd_pool.tile([128, 1024], BF16, name=f"exd{u}")
               for u in range(NU)]

        # ---------------- emission ----------------
        # Proj phase: weave three streams of attention units between
        # projection sub-bursts as their k-tiles become ready, so ACT (the
        # bottleneck) starts early and rarely starves while PE does the
        # 41us of projection work:
        #   streams 0,1 = (q0,h0) / (q0,h1), acc matmuls inline (2 PSUM accs)
        #   stream  2   = (q1,h0), exp parked in exd[]; its acc matmuls run
        #                 after q0's epilogues free a PSUM accumulator bank.
        for c in range(3):
            dma_chunk(c)
        accs0 = [psB.tile([65, QC], F32, tag="psB", name=f"acc0_{h}")
                 for h in range(2)]
        next_u = [0, 0, 0]

        def emit_units(u_lim, n_max=NU * 3, s2_lim=-1):
            # s2 (the deferred q1/h0 stream) is gated until QT2 chunk 1 is
            # projected (emitted at the start of proj chunk 1)
            lims = [u_lim, u_lim, min(u_lim, s2_lim)]
            n = 0
            while n < n_max and any(next_u[s] <= lims[s] for s in range(3)):
                for s in range(3):
                    if next_u[s] <= lims[s] and n < n_max:
                        if s < 2:
                            attn_unit(0, s, next_u[s], accs0[s])
                        else:
                            attn_scexp(1, 0, next_u[2], exd[next_u[2]])
                        next_u[s] += 1
                        n += 1

        for c in range(NQ):
            if c + 3 < NQ:
                dma_chunk(c + 3)
            proj_qk(wk_sb, KT2, c)
            if c == 0:
                proj_qk(wq_sb, QT2, 0)
            if c == 1:
                proj_qk(wq_sb, QT2, 1)
            for st in range(4):
                proj_v_tile(c, st)
                # unit u needs va k-tiles 2u..2u+1, i.e. all tiles <= 4c+st
                u_lim = (4 * c + st - 1) // 2
                emit_units(u_lim, NU * 3 if st == 3 else 2,
                           s2_lim=NU if c >= 2 else -1)
        emit_units(NU - 1, s2_lim=NU)
        epilogue(0, 0, accs0[0])
        epilogue(0, 1, accs0[1])

        # resolution: q1/h0's deferred acc matmuls, woven with q1/h1's units
        # (which keep ACT busy); Q proj for chunk 2 slips in as two bursts.
        accs10 = psB.tile([65, QC], F32, tag="psB", name="acc1_0")
        accs11 = psB.tile([65, QC], F32, tag="psB", name="acc1_1")
        qps = None
        for u in range(NU):
            attn_acc(0, u, exd[u], accs10)
            attn_unit(1, 1, u, accs11)
            if u == 3:
                qps = proj_qk(wq_sb, QT2, 2, t_range=range(0, 4))
            elif u == 4:
                proj_qk(wq_sb, QT2, 2, ps=qps, t_range=range(4, 8))
        epilogue(1, 0, accs10)
        epilogue(1, 1, accs11)

        # steady phase: (qc, h) blocks; Q proj for qc+1 in two 4-mm bursts
        for qc in range(2, NQ):
            for h in range(2):
                accs = psB.tile([65, QC], F32, tag="psB", name=f"acc{qc}_{h}")
                qps = None
                for u in range(NU):
                    attn_unit(qc, h, u, accs)
                    if h == 1 and qc < NQ - 1:
                        if u == 3:
                            qps = proj_qk(wq_sb, QT2, qc + 1, t_range=range(0, 4))
                        elif u == 4:
                            proj_qk(wq_sb, QT2, qc + 1, ps=qps,
                                    t_range=range(4, 8))
                epilogue(qc, h, accs)

    nc.compile()
    return nc


_CACHE: dict = {}


def _get_kernel(with_mask: bool):
    if with_mask not in _CACHE:
        _CACHE[with_mask] = _build_kernel(with_mask)
    return _CACHE[with_mask]


def _bf16(a):
    import ml_dtypes
    return np.ascontiguousarray(a).astype(ml_dtypes.bfloat16)


def build_in_maps(x, w_qkv, maskT=None):
    xTb = _bf16(x[0].T)                                    # [DM, S]
    scale = np.float32(1.0 / np.sqrt(E))
    in_maps = []
    for c in range(NCORES):
        m = {
            "xT": xTb,
            "wq": _bf16(w_qkv[:, EC * c:EC * (c + 1)] * scale),
            "wk": _bf16(w_qkv[:, DM + EC * c:DM + EC * (c + 1)]),
            "wv": _bf16(w_qkv[:, 2 * DM + EC * c:2 * DM + EC * (c + 1)]),
        }
        if maskT is not None:
            m["maskT"] = maskT
        in_maps.append(m)
    return in_maps


def kernel(x: np.ndarray, mask: np.ndarray, w_qkv: np.ndarray) -> np.ndarray:
    x = np.asarray(x, dtype=np.float32)
    mask = np.asarray(mask, dtype=np.float32)
    w_qkv = np.asarray(w_qkv, dtype=np.float32)
    assert x.shape == (1, S, DM) and w_qkv.shape == (DM, 3 * DM)

    with_mask = bool(np.any(mask))
    nc = _get_kernel(with_mask)

    maskT = None
    if with_mask:
        maskT = _bf16(np.broadcast_to(mask, (1, 1, S, S))[0, 0].T)
    in_maps = build_in_maps(x, w_qkv, maskT)

    res = run_bass_kernel_spmd(nc, in_maps, core_ids=list(range(NCORES)))
    # host-side normalize (softmax denominator is row 64/129) and transpose
    outs = []
    for c in range(NCORES):
        o = res.results[c]["outT"]                       # [130, S]
        h0 = o[0:64] / o[64:65]
        h1 = o[65:129] / o[129:130]
        outs.append(np.concatenate([h0, h1], axis=0).T)  # [S, 128]
    return np.ascontiguousarray(
        np.concatenate(outs, axis=1), dtype=np.float32).reshape(1, S, DM)


# revision 19
# speedup vs baseline: 1.1298x; 1.0083x over previous
"""Trainium2 Bass kernel for nn_Attention_9887014715893.

Multi-head attention forward (B=1, S=4096, D=1024, H=16, E=64, fp32):
    qkv = x @ w_qkv ; q,k,v per head ; softmax(q k^T / 8 + mask) @ v

Sharding: tensor-parallel over heads. 8 cores x 2 heads each. Each core gets
the full x (transposed + bf16-cast on host) and its own 128-column bf16
slices of w_qkv, and produces out[:, 128c:128c+128]. No collectives.

Engine budget per core: 33.5M exps keep ACT busy ~264us (the bottleneck);
PE needs ~260us of bf16 matmul streaming (fp32 matmuls would run the PE at
the 1.2GHz "others" clock - bf16 runs 2.4GHz). So everything is organized
around keeping ACT 100% fed:
  - all matmul operands bf16 (PSUM accumulation fp32); host pre-casts inputs
  - scores tiles are [128, 1024] fp32 (2 PSUM banks; 3 slots + 2 acc banks
    = 8 banks exactly): each ACTIVATE covers 1024 elements and the 3-slot
    ring gives ACT a ~2.2us lookahead buffer over PE hiccups
  - x/w DMAs split across both HWDGE queues (sync + scalar)
  - projection is woven into early attention units at sub-microsecond
    granularity so ACT starts ~16us in and rarely starves

Layouts: QT2/KT2 [128, 4096] bf16 (two heads stacked on partitions,
1/sqrt(E) folded into wq on host). V projected directly transposed
(out[s,e] = x-tile^T @ wv) into va [128, 65*32] bf16 with a ones column so
the softmax denominator falls out of the attn@V matmul as row 64. Scores
kept transposed (k on partitions, q free). Epilogue: raw [65, q]
accumulators DMA'd to HBM; the divide by the denominator row and the final
[e,s]->[s,e] transpose happen on the host during the gather.
"""

import sys

if "/opt/trn_rl_repo" not in sys.path:
    sys.path.insert(0, "/opt/trn_rl_repo")

import numpy as np
from contextlib import ExitStack

import concourse.bass as bass
import concourse.bacc as bacc
import concourse.tile as tile
import concourse.mybir as mybir
from concourse.bass_utils import run_bass_kernel_spmd

F32 = mybir.dt.float32
BF16 = mybir.dt.bfloat16
EXP = mybir.ActivationFunctionType.Exp

S = 4096          # sequence length
DM = 1024         # model dim
E = 64            # head dim
NCORES = 8
EC = 128          # output columns per core (2 heads x 64)
QC = 512          # q chunk (free axis of transposed scores)
NQ = S // QC      # 8 q chunks
NK = S // 128     # 32 k tiles
ND = DM // 128    # 8 d tiles
# attention "units": u 0..15 cover 2 k-tiles each
NU = 16


def _unit_kts(u):
    return range(2 * u, 2 * u + 2)


def _build_kernel(with_mask: bool):
    nc = bacc.Bacc("TRN2", target_bir_lowering=False, debug=False,
                   enable_asserts=False, num_devices=NCORES)
    xT = nc.dram_tensor("xT", [DM, S], BF16, kind="ExternalInput").ap()
    wq = nc.dram_tensor("wq", [DM, EC], BF16, kind="ExternalInput").ap()
    wk = nc.dram_tensor("wk", [DM, EC], BF16, kind="ExternalInput").ap()
    wv = nc.dram_tensor("wv", [DM, EC], BF16, kind="ExternalInput").ap()
    if with_mask:
        maskT = nc.dram_tensor("maskT", [S, S], BF16, kind="ExternalInput").ap()
    # raw transposed output: rows 0-64 head0 {outT | denom}, 65-129 head1.
    outT = nc.dram_tensor("outT", [130, S], F32, kind="ExternalOutput").ap()

    with tile.TileContext(nc) as tc, ExitStack() as ctx:
        w_pool = ctx.enter_context(tc.tile_pool(name="w", bufs=1))
        wq_sb = w_pool.tile([128, DM], BF16)
        wk_sb = w_pool.tile([128, DM], BF16)
        wv_sb = w_pool.tile([128, DM], BF16)
        # one 3D-AP DMA per weight: dst[p, 128t+j] = w[128t+p, j]
        # (k first on sync, v on the scalar queue -> K proj starts earliest)
        for eng, wsb, w in ((nc.sync, wk_sb, wk), (nc.scalar, wv_sb, wv),
                            (nc.sync, wq_sb, wq)):
            eng.dma_start(
                wsb[:].rearrange("p (t j) -> p t j", t=ND),
                w.rearrange("(t p) j -> p t j", t=ND))

        qt_pool = ctx.enter_context(tc.tile_pool(name="qt", bufs=1))
        QT2 = qt_pool.tile([128, S], BF16)   # rows 0-63 head0 e-dims, 64-127 head1
        KT2 = qt_pool.tile([128, S], BF16)
        va_pool = ctx.enter_context(tc.tile_pool(name="va", bufs=1))
        va = [va_pool.tile([128, 65 * NK], BF16, name=f"va{h}") for h in range(2)]
        ones_b = va_pool.tile([128, 1], BF16)
        nc.vector.memset(ones_b[:], 1.0)
        for h in range(2):
            nc.vector.tensor_copy(va[h][:, 64:65 * NK:65],
                                  ones_b[:].to_broadcast([128, NK]))

        # full x resident in SBUF (8 chunks x [128, 8*512] bf16 = 64KB/par)
        xs_pool = ctx.enter_context(tc.tile_pool(name="xs", bufs=1))
        xs = [xs_pool.tile([128, ND * QC], BF16, name=f"xs{c}") for c in range(NQ)]

        # PSUM: psA 3 slots x 2 banks (scores + proj psums), psB 2 x 1 bank
        psA = ctx.enter_context(tc.tile_pool(name="psA", bufs=3, space="PSUM"))
        psB = ctx.enter_context(tc.tile_pool(name="psB", bufs=2, space="PSUM"))

        exp_pool = ctx.enter_context(tc.tile_pool(name="exp", bufs=8))
        accsb_pool = ctx.enter_context(tc.tile_pool(name="accsb", bufs=4))
        if with_mask:
            msk_pool = ctx.enter_context(tc.tile_pool(name="msk", bufs=3))

        def dma_chunk(c):
            # alternate d-tiles across the two HWDGE queues (sync / scalar)
            s0 = QC * c
            for t in range(ND):
                eng = nc.sync if t % 2 == 0 else nc.scalar
                eng.dma_start(xs[c][:, QC * t:QC * (t + 1)],
                              xT[128 * t:128 * (t + 1), s0:s0 + QC])

        def proj_qk(wsb, dst, c, ps=None, t_range=None):
            # t_range splits the 8 accumulation matmuls across call sites
            # (sub-1.7us PE bursts keep ACT's short psA runway alive); pass
            # the returned psum tile to the continuation call.
            s0 = QC * c
            if ps is None:
                ps = psA.tile([128, QC], F32, tag="psA")
            ts = list(range(ND)) if t_range is None else list(t_range)
            for t in ts:
                nc.tensor.matmul(ps[:], lhsT=wsb[:, 128 * t:128 * (t + 1)],
                                 rhs=xs[c][:, QC * t:QC * (t + 1)],
                                 start=(t == 0), stop=(t == ND - 1))
            if ts[-1] == ND - 1:
                nc.vector.tensor_copy(dst[:, s0:s0 + QC], ps[:])
            return ps

        def proj_v_tile(c, st):
            # direct transposed V: out[s-tile, e] = sum_t x-tile^T @ wv-tile
            kk = 4 * c + st
            ps = psA.tile([128, 128], F32, tag="psA")
            for t in range(ND):
                nc.tensor.matmul(
                    ps[:],
                    lhsT=xs[c][:, QC * t + 128 * st:QC * t + 128 * (st + 1)],
                    rhs=wv_sb[:, 128 * t:128 * (t + 1)],
                    start=(t == 0), stop=(t == ND - 1))
            nc.vector.tensor_copy(va[0][:, 65 * kk:65 * kk + 64], ps[:, 0:64])
            nc.vector.tensor_copy(va[1][:, 65 * kk:65 * kk + 64], ps[:, 64:128])

        # ---- attention unit: 3 (or 2) k-tiles for (qc, h), one ACTIVATE ----
        def attn_scexp(qc, h, u, ex):
            q0 = QC * qc
            kts = list(_unit_kts(u))
            w = 512 * len(kts)
            if with_mask:
                msk = msk_pool.tile([128, 1024], BF16, tag="msk")
                for j, kt in enumerate(kts):
                    nc.sync.dma_start(
                        msk[:, 512 * j:512 * (j + 1)],
                        maskT[128 * kt:128 * (kt + 1), q0:q0 + QC])
            sc = psA.tile([128, 1024], F32, tag="psA", name=f"sc{qc}_{h}_{u}")
            for j, kt in enumerate(kts):
                nc.tensor.matmul(
                    sc[:, 512 * j:512 * (j + 1)],
                    lhsT=KT2[64 * h:64 * (h + 1), 128 * kt:128 * (kt + 1)],
                    rhs=QT2[64 * h:64 * (h + 1), q0:q0 + QC],
                    start=True, stop=True,
                    tile_position=(64 * h, 0),
                )
            if with_mask:
                nc.vector.tensor_tensor(out=sc[:, 0:w], in0=sc[:, 0:w],
                                        in1=msk[:, 0:w], op=mybir.AluOpType.add)
            nc.scalar.activation(ex[:, 0:w], sc[:, 0:w], EXP)

        def attn_acc(h, u, ex, accs):
            for j, kt in enumerate(_unit_kts(u)):
                nc.tensor.matmul(
                    accs[:],
                    lhsT=va[h][:, 65 * kt:65 * kt + 65],
                    rhs=ex[:, 512 * j:512 * (j + 1)],
                    start=(kt == 0), stop=(kt == NK - 1),
                )

        def attn_unit(qc, h, u, accs):
            ex = exp_pool.tile([128, 1024], BF16, tag="exp", name=f"ex{qc}_{h}_{u}")
            attn_scexp(qc, h, u, ex)
            attn_acc(h, u, ex, accs)

        def epilogue(qc, h, accs):
            asb = accsb_pool.tile([65, QC], F32, tag="accsb")
            nc.vector.tensor_copy(asb[:], accs[:])
            nc.sync.dma_start(outT[65 * h:65 * h + 65, QC * qc:QC * (qc + 1)], asb[:])

        # deferred-exp store for q1/h0's units computed during the proj phase
        exd_pool = ctx.enter_context(tc.tile_pool(name="exd", bufs=1))
        exd = [exd_pool.tile([128, 1024], BF16, name=f"exd{u}")
               for u in range(NU)]

        # ---------------- emission ----------------
        # Proj phase: weave three streams of attention units between
        # projection sub-bursts as their k-tiles become ready, so ACT (the
        # bottleneck) starts early and rarely starves while PE does the
        # 41us of projection work:
        #   streams 0,1 = (q0,h0) / (q0,h1), acc matmuls inline (2 PSUM accs)
        #   stream  2   = (q1,h0), exp parked in exd[]; its acc matmuls run
        #                 after q0's epilogues free a PSUM accumulator bank.
        for c in range(3):
            dma_chunk(c)
        accs0 = [psB.tile([65, QC], F32, tag="psB", name=f"acc0_{h}")
                 for h in range(2)]
        next_u = [0, 0, 0]

        def emit_units(u_lim, n_max=NU * 3, s2_lim=-1):
            # s2 (the deferred q1/h0 stream) is gated until QT2 chunk 1 is
            # projected (emitted at the start of proj chunk 1)
            lims = [u_lim, u_lim, min(u_lim, s2_lim)]
            n = 0
            while n < n_max and any(next_u[s] <= lims[s] for s in range(3)):
                for s in range(3):
                    if next_u[s] <= lims[s] and n < n_max:
                        if s < 2:
                            attn_unit(0, s, next_u[s], accs0[s])
                        else:
                            attn_scexp(1, 0, next_u[2], exd[next_u[2]])
                        next_u[s] += 1
                        n += 1

        for c in range(NQ):
            if c + 3 < NQ:
                dma_chunk(c + 3)
            proj_qk(wk_sb, KT2, c)
            if c == 0:
                proj_qk(wq_sb, QT2, 0)
            if c == 1:
                proj_qk(wq_sb, QT2, 1)
            for st in range(4):
                proj_v_tile(c, st)
                # unit u needs va k-tiles 2u..2u+1, i.e. all tiles <= 4c+st
                u_lim = (4 * c + st - 1) // 2
                emit_units(u_lim, NU * 3 if st == 3 else 2,
                           s2_lim=NU if c >= 2 else -1)
        emit_units(NU - 1, s2_lim=NU)
        epilogue(0, 0, accs0[0])
        epilogue(0, 1, accs0[1])

        # resolution: q1/h0's deferred acc matmuls, woven with q1/h1's units
        # (which keep ACT busy); Q proj for chunk 2 slips in as two bursts.
        accs10 = psB.tile([65, QC], F32, tag="psB", name="acc1_0")
        accs11 = psB.tile([65, QC], F32, tag="psB", name="acc1_1")
        qps = None
        for u in range(NU):
            attn_acc(0, u, exd[u], accs10)
            attn_unit(1, 1, u, accs11)
            if u == 3:
                qps = proj_qk(wq_sb, QT2, 2, t_range=range(0, 4))
            elif u == 4:
                proj_qk(wq_sb, QT2, 2, ps=qps, t_range=range(4, 8))
        epilogue(1, 0, accs10)
        epilogue(1, 1, accs11)

        # steady phase: (qc, h) blocks; Q proj for qc+1 in two 4-mm bursts
        for qc in range(2, NQ):
            for h in range(2):
                accs = psB.tile([65, QC], F32, tag="psB", name=f"acc{qc}_{h}")
                for u in range(NU):
                    attn_unit(qc, h, u, accs)
                    if h == 1 and u == 4 and qc < NQ - 1:
                        # Q proj psum from the psB pool (a full PSUM bank
                        # physically) so the scores ring keeps all 3 slots
                        # and ACT rides through this 1.7us PE burst
                        qps = psB.tile([128, QC], F32, tag="psB",
                                       name=f"qproj{qc + 1}")
                        proj_qk(wq_sb, QT2, qc + 1, ps=qps)
                epilogue(qc, h, accs)

    nc.compile()
    return nc


_CACHE: dict = {}


def _get_kernel(with_mask: bool):
    if with_mask not in _CACHE:
        _CACHE[with_mask] = _build_kernel(with_mask)
    return _CACHE[with_mask]


def _bf16(a):
    import ml_dtypes
    return np.ascontiguousarray(a).astype(ml_dtypes.bfloat16)


def build_in_maps(x, w_qkv, maskT=None):
    xTb = _bf16(x[0].T)                                    # [DM, S]
    scale = np.float32(1.0 / np.sqrt(E))
    in_maps = []
    for c in range(NCORES):
        m = {
            "xT": xTb,
            "wq": _bf16(w_qkv[:, EC * c:EC * (c + 1)] * scale),
            "wk": _bf16(w_qkv[:, DM + EC * c:DM + EC * (c + 1)]),
            "wv": _bf16(w_qkv[:, 2 * DM + EC * c:2 * DM + EC * (c + 1)]),
        }
        if maskT is not None:
            m["maskT"] = maskT
        in_maps.append(m)
    return in_maps


def kernel(x: np.ndarray, mask: np.ndarray, w_qkv: np.ndarray) -> np.ndarray:
    x = np.asarray(x, dtype=np.float32)
    mask = np.asarray(mask, dtype=np.float32)
    w_qkv = np.asarray(w_qkv, dtype=np.float32)
    assert x.shape == (1, S, DM) and w_qkv.shape == (DM, 3 * DM)

    with_mask = bool(np.any(mask))
    nc = _get_kernel(with_mask)

    maskT = None
    if with_mask:
        maskT = _bf16(np.broadcast_to(mask, (1, 1, S, S))[0, 0].T)
    in_maps = build_in_maps(x, w_qkv, maskT)

    res = run_bass_kernel_spmd(nc, in_maps, core_ids=list(range(NCORES)))
    # host-side normalize (softmax denominator is row 64/129) and transpose
    outs = []
    for c in range(NCORES):
        o = res.results[c]["outT"]                       # [130, S]
        h0 = o[0:64] / o[64:65]
        h1 = o[65:129] / o[129:130]
        outs.append(np.concatenate([h0, h1], axis=0).T)  # [S, 128]
    return np.ascontiguousarray(
        np.concatenate(outs, axis=1), dtype=np.float32).reshape(1, S, DM)


# revision 21
# speedup vs baseline: 1.1354x; 1.0049x over previous
"""Trainium2 Bass kernel for nn_Attention_9887014715893.

Multi-head attention forward (B=1, S=4096, D=1024, H=16, E=64, fp32):
    qkv = x @ w_qkv ; q,k,v per head ; softmax(q k^T / 8 + mask) @ v

Sharding: tensor-parallel over heads. 8 cores x 2 heads each. Each core gets
the full x (transposed + bf16-cast on host) and its own 128-column bf16
slices of w_qkv, and produces out[:, 128c:128c+128]. No collectives.

Engine budget per core: 33.5M exps keep ACT busy ~264us (the bottleneck);
PE needs ~260us of bf16 matmul streaming (fp32 matmuls would run the PE at
the 1.2GHz "others" clock - bf16 runs 2.4GHz). So everything is organized
around keeping ACT 100% fed:
  - all matmul operands bf16 (PSUM accumulation fp32); host pre-casts inputs
  - scores tiles are [128, 1024] fp32 (2 PSUM banks; 3 slots + 2 acc banks
    = 8 banks exactly): each ACTIVATE covers 1024 elements and the 3-slot
    ring gives ACT a ~2.2us lookahead buffer over PE hiccups
  - x/w DMAs split across both HWDGE queues (sync + scalar)
  - projection is woven into early attention units at sub-microsecond
    granularity so ACT starts ~16us in and rarely starves

Layouts: QT2/KT2 [128, 4096] bf16 (two heads stacked on partitions,
1/sqrt(E) folded into wq on host). V projected directly transposed
(out[s,e] = x-tile^T @ wv) into va [128, 65*32] bf16 with a ones column so
the softmax denominator falls out of the attn@V matmul as row 64. Scores
kept transposed (k on partitions, q free). Epilogue: raw [65, q]
accumulators DMA'd to HBM; the divide by the denominator row and the final
[e,s]->[s,e] transpose happen on the host during the gather.
"""

import sys

if "/opt/trn_rl_repo" not in sys.path:
    sys.path.insert(0, "/opt/trn_rl_repo")

import numpy as np
from contextlib import ExitStack

import concourse.bass as bass
import concourse.bacc as bacc
import concourse.tile as tile
import concourse.mybir as mybir
from concourse.bass_utils import run_bass_kernel_spmd

F32 = mybir.dt.float32
BF16 = mybir.dt.bfloat16
EXP = mybir.ActivationFunctionType.Exp

S = 4096          # sequence length
DM = 1024         # model dim
E = 64            # head dim
NCORES = 8
EC = 128          # output columns per core (2 heads x 64)
QC = 512          # q chunk (free axis of transposed scores)
NQ = S // QC      # 8 q chunks
NK = S // 128     # 32 k tiles
ND = DM // 128    # 8 d tiles
# attention "units": u 0..15 cover 2 k-tiles each
NU = 16


def _unit_kts(u):
    return range(2 * u, 2 * u + 2)


def _build_kernel(with_mask: bool):
    nc = bacc.Bacc("TRN2", target_bir_lowering=False, debug=False,
                   enable_asserts=False, num_devices=NCORES)
    xT = nc.dram_tensor("xT", [DM, S], BF16, kind="ExternalInput").ap()
    wq = nc.dram_tensor("wq", [DM, EC], BF16, kind="ExternalInput").ap()
    wk = nc.dram_tensor("wk", [DM, EC], BF16, kind="ExternalInput").ap()
    wv = nc.dram_tensor("wv", [DM, EC], BF16, kind="ExternalInput").ap()
    if with_mask:
        maskT = nc.dram_tensor("maskT", [S, S], BF16, kind="ExternalInput").ap()
    # raw transposed output: rows 0-64 head0 {outT | denom}, 65-129 head1.
    outT = nc.dram_tensor("outT", [130, S], F32, kind="ExternalOutput").ap()

    with tile.TileContext(nc) as tc, ExitStack() as ctx:
        w_pool = ctx.enter_context(tc.tile_pool(name="w", bufs=1))
        wq_sb = w_pool.tile([128, DM], BF16)
        wk_sb = w_pool.tile([128, DM], BF16)
        wv_sb = w_pool.tile([128, DM], BF16)
        # one 3D-AP DMA per weight: dst[p, 128t+j] = w[128t+p, j]
        # (k first on sync, v on the scalar queue -> K proj starts earliest)
        for eng, wsb, w in ((nc.sync, wk_sb, wk), (nc.scalar, wv_sb, wv),
                            (nc.sync, wq_sb, wq)):
            eng.dma_start(
                wsb[:].rearrange("p (t j) -> p t j", t=ND),
                w.rearrange("(t p) j -> p t j", t=ND))

        qt_pool = ctx.enter_context(tc.tile_pool(name="qt", bufs=1))
        QT2 = qt_pool.tile([128, S], BF16)   # rows 0-63 head0 e-dims, 64-127 head1
        KT2 = qt_pool.tile([128, S], BF16)
        va_pool = ctx.enter_context(tc.tile_pool(name="va", bufs=1))
        va = [va_pool.tile([128, 65 * NK], BF16, name=f"va{h}") for h in range(2)]
        ones_b = va_pool.tile([128, 1], BF16)
        nc.vector.memset(ones_b[:], 1.0)
        for h in range(2):
            nc.vector.tensor_copy(va[h][:, 64:65 * NK:65],
                                  ones_b[:].to_broadcast([128, NK]))

        # full x resident in SBUF (8 chunks x [128, 8*512] bf16 = 64KB/par)
        xs_pool = ctx.enter_context(tc.tile_pool(name="xs", bufs=1))
        xs = [xs_pool.tile([128, ND * QC], BF16, name=f"xs{c}") for c in range(NQ)]

        # PSUM: psA 3 slots x 2 banks (scores + proj psums), psB 2 x 1 bank
        psA = ctx.enter_context(tc.tile_pool(name="psA", bufs=3, space="PSUM"))
        psB = ctx.enter_context(tc.tile_pool(name="psB", bufs=2, space="PSUM"))

        exp_pool = ctx.enter_context(tc.tile_pool(name="exp", bufs=8))
        accsb_pool = ctx.enter_context(tc.tile_pool(name="accsb", bufs=4))
        if with_mask:
            msk_pool = ctx.enter_context(tc.tile_pool(name="msk", bufs=3))

        def dma_chunk(c):
            # alternate d-tiles across the two HWDGE queues (sync / scalar)
            s0 = QC * c
            for t in range(ND):
                eng = nc.sync if t % 2 == 0 else nc.scalar
                eng.dma_start(xs[c][:, QC * t:QC * (t + 1)],
                              xT[128 * t:128 * (t + 1), s0:s0 + QC])

        def proj_qk(wsb, dst, c, ps=None, t_range=None):
            # t_range splits the 8 accumulation matmuls across call sites
            # (sub-1.7us PE bursts keep ACT's short psA runway alive); pass
            # the returned psum tile to the continuation call.
            s0 = QC * c
            if ps is None:
                ps = psA.tile([128, QC], F32, tag="psA")
            ts = list(range(ND)) if t_range is None else list(t_range)
            for t in ts:
                nc.tensor.matmul(ps[:], lhsT=wsb[:, 128 * t:128 * (t + 1)],
                                 rhs=xs[c][:, QC * t:QC * (t + 1)],
                                 start=(t == 0), stop=(t == ND - 1))
            if ts[-1] == ND - 1:
                nc.vector.tensor_copy(dst[:, s0:s0 + QC], ps[:])
            return ps

        def proj_v_tile(c, st):
            # direct transposed V: out[s-tile, e] = sum_t x-tile^T @ wv-tile
            kk = 4 * c + st
            ps = psA.tile([128, 128], F32, tag="psA")
            for t in range(ND):
                nc.tensor.matmul(
                    ps[:],
                    lhsT=xs[c][:, QC * t + 128 * st:QC * t + 128 * (st + 1)],
                    rhs=wv_sb[:, 128 * t:128 * (t + 1)],
                    start=(t == 0), stop=(t == ND - 1))
            nc.vector.tensor_copy(va[0][:, 65 * kk:65 * kk + 64], ps[:, 0:64])
            nc.vector.tensor_copy(va[1][:, 65 * kk:65 * kk + 64], ps[:, 64:128])

        # ---- attention unit: 3 (or 2) k-tiles for (qc, h), one ACTIVATE ----
        def attn_scexp(qc, h, u, ex):
            q0 = QC * qc
            kts = list(_unit_kts(u))
            w = 512 * len(kts)
            if with_mask:
                msk = msk_pool.tile([128, 1024], BF16, tag="msk")
                for j, kt in enumerate(kts):
                    nc.sync.dma_start(
                        msk[:, 512 * j:512 * (j + 1)],
                        maskT[128 * kt:128 * (kt + 1), q0:q0 + QC])
            sc = psA.tile([128, 1024], F32, tag="psA", name=f"sc{qc}_{h}_{u}")
            for j, kt in enumerate(kts):
                nc.tensor.matmul(
                    sc[:, 512 * j:512 * (j + 1)],
                    lhsT=KT2[64 * h:64 * (h + 1), 128 * kt:128 * (kt + 1)],
                    rhs=QT2[64 * h:64 * (h + 1), q0:q0 + QC],
                    start=True, stop=True,
                    tile_position=(64 * h, 0),
                )
            if with_mask:
                nc.vector.tensor_tensor(out=sc[:, 0:w], in0=sc[:, 0:w],
                                        in1=msk[:, 0:w], op=mybir.AluOpType.add)
            nc.scalar.activation(ex[:, 0:w], sc[:, 0:w], EXP)

        def attn_acc(h, u, ex, accs):
            for j, kt in enumerate(_unit_kts(u)):
                nc.tensor.matmul(
                    accs[:],
                    lhsT=va[h][:, 65 * kt:65 * kt + 65],
                    rhs=ex[:, 512 * j:512 * (j + 1)],
                    start=(kt == 0), stop=(kt == NK - 1),
                )

        def attn_unit(qc, h, u, accs):
            ex = exp_pool.tile([128, 1024], BF16, tag="exp", name=f"ex{qc}_{h}_{u}")
            attn_scexp(qc, h, u, ex)
            attn_acc(h, u, ex, accs)

        def epilogue(qc, h, accs):
            asb = accsb_pool.tile([65, QC], F32, tag="accsb")
            nc.vector.tensor_copy(asb[:], accs[:])
            nc.sync.dma_start(outT[65 * h:65 * h + 65, QC * qc:QC * (qc + 1)], asb[:])

        # deferred-exp store for q1/h0's units computed during the proj phase
        exd_pool = ctx.enter_context(tc.tile_pool(name="exd", bufs=1))
        exd = [exd_pool.tile([128, 1024], BF16, name=f"exd{u}")
               for u in range(NU)]

        # ---------------- emission ----------------
        # Proj phase: weave three streams of attention units between
        # projection sub-bursts as their k-tiles become ready, so ACT (the
        # bottleneck) starts early and rarely starves while PE does the
        # 41us of projection work:
        #   streams 0,1 = (q0,h0) / (q0,h1), acc matmuls inline (2 PSUM accs)
        #   stream  2   = (q1,h0), exp parked in exd[]; its acc matmuls run
        #                 after q0's epilogues free a PSUM accumulator bank.
        for c in range(3):
            dma_chunk(c)
        accs0 = [psB.tile([65, QC], F32, tag="psB", name=f"acc0_{h}")
                 for h in range(2)]
        next_u = [0, 0, 0]
        pend = [None, None]  # per inline stream: (u, ex) awaiting its accs

        def emit_units(u_lim, n_max=NU * 3, s2_lim=-1):
            # s2 (the deferred q1/h0 stream) is gated until QT2 chunk 1 is
            # projected (emitted at the start of proj chunk 1).
            # Inline streams are software-pipelined: the acc matmuls of unit
            # u-1 are emitted after unit u's scores/exp, so PE never sits at
            # an acc matmul waiting on ACT while proj work queues behind it.
            lims = [u_lim, u_lim, min(u_lim, s2_lim)]
            n = 0
            while n < n_max and any(next_u[s] <= lims[s] for s in range(3)):
                for s in range(3):
                    if next_u[s] <= lims[s] and n < n_max:
                        u = next_u[s]
                        if s < 2:
                            ex = exp_pool.tile([128, 1024], BF16, tag="exp",
                                               name=f"ex0_{s}_{u}")
                            attn_scexp(0, s, u, ex)
                            if pend[s] is not None:
                                pu, pex = pend[s]
                                attn_acc(s, pu, pex, accs0[s])
                            pend[s] = (u, ex)
                        else:
                            attn_scexp(1, 0, u, exd[u])
                        next_u[s] += 1
                        n += 1

        for c in range(NQ):
            if c + 3 < NQ:
                dma_chunk(c + 3)
            proj_qk(wk_sb, KT2, c)
            if c == 0:
                proj_qk(wq_sb, QT2, 0)
            if c == 1:
                proj_qk(wq_sb, QT2, 1)
            for st in range(4):
                proj_v_tile(c, st)
                # unit u needs va k-tiles 2u..2u+1, i.e. all tiles <= 4c+st
                u_lim = (4 * c + st - 1) // 2
                emit_units(u_lim, NU * 3 if st == 3 else 2,
                           s2_lim=NU if c >= 2 else -1)
        emit_units(NU - 1, s2_lim=NU)
        for s in range(2):
            pu, pex = pend[s]
            attn_acc(s, pu, pex, accs0[s])
        epilogue(0, 0, accs0[0])
        epilogue(0, 1, accs0[1])

        # resolution: q1/h0's deferred acc matmuls, woven with q1/h1's units
        # (sc/exp pipelined 2 ahead of accs); Q proj chunk 2 as two bursts.
        accs10 = psB.tile([65, QC], F32, tag="psB", name="acc1_0")
        accs11 = psB.tile([65, QC], F32, tag="psB", name="acc1_1")
        exq = []

        def scexp11(u):
            ex = exp_pool.tile([128, 1024], BF16, tag="exp", name=f"ex1_1_{u}")
            attn_scexp(1, 1, u, ex)
            exq.append(ex)

        scexp11(0)
        scexp11(1)
        qps = None
        for u in range(NU):
            if u + 2 < NU:
                scexp11(u + 2)
            attn_acc(0, u, exd[u], accs10)
            attn_acc(1, u, exq.pop(0), accs11)
            if u == 3:
                qps = proj_qk(wq_sb, QT2, 2, t_range=range(0, 4))
            elif u == 4:
                proj_qk(wq_sb, QT2, 2, ps=qps, t_range=range(4, 8))
        epilogue(1, 0, accs10)
        epilogue(1, 1, accs11)

        # steady phase: one flat software-pipelined stream of units across
        # all remaining (qc, h) blocks — scores/exp run 2 units ahead of the
        # acc matmuls so ACT always has ~2 exps of runway, riding through
        # Q-proj bursts (whose psums live in a psB bank, not the sc ring).
        units = [(qc, h, u) for qc in range(2, NQ)
                 for h in range(2) for u in range(NU)]
        accs_map = {}
        exq2 = []

        def scexp_st(i):
            qc, h, u = units[i]
            ex = exp_pool.tile([128, 1024], BF16, tag="exp",
                               name=f"ex{qc}_{h}_{u}")
            attn_scexp(qc, h, u, ex)
            exq2.append(ex)

        scexp_st(0)
        scexp_st(1)
        for i, (qc, h, u) in enumerate(units):
            if i + 2 < len(units):
                scexp_st(i + 2)
            if u == 0:
                accs_map[(qc, h)] = psB.tile([65, QC], F32, tag="psB",
                                             name=f"acc{qc}_{h}")
            attn_acc(h, u, exq2.pop(0), accs_map[(qc, h)])
            if u == NU - 1:
                epilogue(qc, h, accs_map.pop((qc, h)))
            if h == 1 and u == 4 and qc < NQ - 1:
                qps = psB.tile([128, QC], F32, tag="psB",
                               name=f"qproj{qc + 1}")
                proj_qk(wq_sb, QT2, qc + 1, ps=qps)

    nc.compile()
    return nc


_CACHE: dict = {}


def _get_kernel(with_mask: bool):
    if with_mask not in _CACHE:
        _CACHE[with_mask] = _build_kernel(with_mask)
    return _CACHE[with_mask]


def _bf16(a):
    import ml_dtypes
    return np.ascontiguousarray(a).astype(ml_dtypes.bfloat16)


def build_in_maps(x, w_qkv, maskT=None):
    xTb = _bf16(x[0].T)                                    # [DM, S]
    scale = np.float32(1.0 / np.sqrt(E))
    in_maps = []
    for c in range(NCORES):
        m = {
            "xT": xTb,
            "wq": _bf16(w_qkv[:, EC * c:EC * (c + 1)] * scale),
            "wk": _bf16(w_qkv[:, DM + EC * c:DM + EC * (c + 1)]),
            "wv": _bf16(w_qkv[:, 2 * DM + EC * c:2 * DM + EC * (c + 1)]),
        }
        if maskT is not None:
            m["maskT"] = maskT
        in_maps.append(m)
    return in_maps


def kernel(x: np.ndarray, mask: np.ndarray, w_qkv: np.ndarray) -> np.ndarray:
    x = np.asarray(x, dtype=np.float32)
    mask = np.asarray(mask, dtype=np.float32)
    w_qkv = np.asarray(w_qkv, dtype=np.float32)
    assert x.shape == (1, S, DM) and w_qkv.shape == (DM, 3 * DM)

    with_mask = bool(np.any(mask))
    nc = _get_kernel(with_mask)

    maskT = None
    if with_mask:
        maskT = _bf16(np.broadcast_to(mask, (1, 1, S, S))[0, 0].T)
    in_maps = build_in_maps(x, w_qkv, maskT)

    res = run_bass_kernel_spmd(nc, in_maps, core_ids=list(range(NCORES)))
    # host-side normalize (softmax denominator is row 64/129) and transpose
    outs = []
    for c in range(NCORES):
        o = res.results[c]["outT"]                       # [130, S]
        h0 = o[0:64] / o[64:65]
        h1 = o[65:129] / o[129:130]
        outs.append(np.concatenate([h0, h1], axis=0).T)  # [S, 128]
    return np.ascontiguousarray(
        np.concatenate(outs, axis=1), dtype=np.float32).reshape(1, S, DM)
